# revision 46
# baseline (speedup 1.0000x reference)
"""Bass/Tile TRN2 kernel for nn_DecoderGroupedQueryHeadAttentionAlibi.

Sharding (8 cores): core = (b, g) with b = core//2 in [0,4) (batch),
g = core%2 (head-half). Each core computes 8 of 16 query heads (those with
h%4 in {2g, 2g+1}) for its batch, plus the corresponding row-slice of the
output projection; the host sums the two half partials and adds bproj.

Per-core device program (scoresT layout = [s_partitions, t_free]):
  - projections q/k/v from host-pretransposed xT/weight tiles (bf16 matmuls)
  - per (head, s-tile j): scoresT psum [128, W] -> ACT exp(S/8) (no bias)
    -> one DVE multiply on [lo, W) with a host-precomputed min-folded
    Toeplitz table mtab[s_in, d] = min(exp(-a*(d - s_in)), 1), d = t - 128j
    (the strict-future region t < 128j needs no fix: its alibi bias is 0)
  - attn@v accumulation in psum [65, 2048] where row 64 (a ones column in
    v) is the softmax denominator
  - per-pair incremental reciprocal+normalize; output projection at the end.

Banding: columns t >= 128(j+1) + cutoff_i are dropped (the decayed alibi
weight there is < exp(-CUT_MARGIN)); widths are 128-granular and shared
across cores (SPMD) via the per-slot max cutoff.
"""

import math
import numpy as np

# ---- problem constants (hardcoded; kernel.py must be self-contained) ----
B, T, C = 4, 2048, 1024
N_HEAD, N_KV_HEAD, HEAD_DIM = 16, 4, 64
NH = 8            # heads per core
ST = T // 128     # 16 s-tiles
KCT = C // 128    # 8 contraction tiles of 128
CUT_MARGIN = 3.5  # exp(-3.5) ~ 3e-2 decay at the band edge; dropped
                  # softmax mass is ~2e-3 of the denominator worst-case

_START = 2.0 ** (-2.0 ** (-(math.log2(N_HEAD) - 3.0)))  # 0.7071...


def _head_of_slot(i: int, g: int) -> int:
    return 4 * (i // 2) + 2 * g + (i % 2)


def _a_of_head(h: int) -> float:
    return (_START ** (h + 1)) / math.sqrt(HEAD_DIM)


# Loop bounds must be identical on every core (SPMD): use the widest cutoff
# over g for each head slot (g=1 heads have smaller slopes -> wider bands).
_CUTOFF = [CUT_MARGIN / min(_a_of_head(_head_of_slot(i, 0)),
                            _a_of_head(_head_of_slot(i, 1)))
           for i in range(NH)]
# 128-granular band width beyond the diagonal tile, and per-(i,j) widths.
_C128 = [min(T, 128 * math.ceil(c / 128.0)) for c in _CUTOFF]
_W = [[min(T, 128 * (j + 1) + _C128[i]) for j in range(ST)] for i in range(NH)]
# mtab slot widths / offsets (table covers d = t - 128j in [0, W - lo))
_MT_W = [min(T, 128 + _C128[i]) for i in range(NH)]
_MT_OFF = [sum(_MT_W[:i]) for i in range(NH)]
_MT_TOT = sum(_MT_W)
# first j contributing to attn@v psum chunk c (512-wide chunks)
_J_FIRST = [[min(j for j in range(ST) if _W[i][j] > 512 * c)
             for c in range(4)] for i in range(NH)]

_NC_CACHE = {}


def _split_multiwait(nc, mybir, max_waits=1):
    """walrus in this env encodes at most one sync-wait per instruction;
    split extras onto same-engine NoOps emitted just before."""
    for f in nc.m.functions:
        for bb in f.blocks:
            new = []
            for ins in bb.instructions:
                si = ins.sync_info
                conds = list(si.on_wait) if si is not None else []
                if len(conds) > max_waits:
                    for cond in conds[:-max_waits]:
                        n = mybir.InstNoOp(
                            name=nc.get_next_instruction_name(), ins=[], outs=[])
                        n.engine = ins.engine
                        n.sync_info = mybir.SyncInfo(on_wait=[cond], on_update=[])
                        new.append(n)
                    si.on_wait = conds[-max_waits:]
                new.append(ins)
            bb.instructions = new


def _build_nc():
    if "nc" in _NC_CACHE:
        return _NC_CACHE["nc"]
    import concourse.bass as bass
    import concourse.tile as tile
    from concourse import mybir

    f32 = mybir.dt.float32
    bf16 = mybir.dt.bfloat16
    AF = mybir.ActivationFunctionType
    MUL = mybir.AluOpType.mult

    nc = bass.Bass()

    xT_d = nc.dram_tensor("xT", [C, T], bf16, kind="ExternalInput")
    wq_d = nc.dram_tensor("wqT", [C, NH * 64], bf16, kind="ExternalInput")
    wk_d = nc.dram_tensor("wkT", [C, 128], bf16, kind="ExternalInput")
    wv_d = nc.dram_tensor("wvT", [C, 128], bf16, kind="ExternalInput")
    wp_d = nc.dram_tensor("wpT", [NH * 64, C], bf16, kind="ExternalInput")
    mt_d = nc.dram_tensor("mtab", [128, _MT_TOT], bf16, kind="ExternalInput")
    out_d = nc.dram_tensor("out", [T, C], f32, kind="ExternalOutput")

    xT_r = xT_d.rearrange("(k p) t -> p k t", p=128)
    wq_r = wq_d.rearrange("(k p) e -> p k e", p=128)

    with tile.TileContext(nc) as tc:
        with (
            tc.tile_pool(name="const", bufs=1) as const,
        ):
            # ---- persistent tiles ----
            mtab = const.tile([128, _MT_TOT], bf16)
            kRep = const.tile([128, 2, T], bf16)     # kv on both halves
            v_sb = const.tile([128, ST, 130], bf16)  # [s, j, (v_kv0|1|v_kv1|1)]
            qRep = const.tile([128, NH, T], bf16)    # head i on both halves
            outT = const.tile([128, 4, T], bf16)     # [(2 heads d), pair, t]
            wp = const.tile([128, 4, C], bf16)
            dstack = const.tile([128, 128], bf16)    # [(head,tt), t_in] denom
            dstf = const.tile([128, 128], f32)
            rstf = const.tile([128, 128], f32)
            rstb = const.tile([128, 128], bf16)

            # ---- phases 1+2 interleaved ----
            with (
                tc.tile_pool(name="ph1", bufs=1) as ph1,
                tc.tile_pool(name="work", bufs=3) as work,
                tc.tile_pool(name="ebuf", bufs=10) as ebufp,
                tc.tile_pool(name="dstgp", bufs=2) as dstgp,
                tc.tile_pool(name="dramd", bufs=2, space="DRAM") as dramd,
                tc.tile_pool(name="dramr", bufs=1, space="DRAM") as dramr,
            ):
              ddrow = dramd.tile([NH, T], bf16)
              rdram = dramr.tile([NH, T], bf16)
              rd3 = rdram.rearrange("i (a b) -> i a b", b=128)
              xT = ph1.tile([128, KCT, T], bf16)
              wk = ph1.tile([128, KCT, 128], bf16)
              wq = ph1.tile([128, KCT, NH * 64], bf16)
              wv = ph1.tile([128, KCT, 128], bf16)
              scr = ph1.tile([128, 640], bf16)

              # k and q-pair-0 projections run in a dedicated 2-bank psum
              # pool that closes before the heads; v and q-pairs 1-3 are
              # emitted just-in-time inside the head loop, borrowing psS
              # rotation slots, so psA's banks are free from head 0 on.
              with tc.tile_pool(name="psq", bufs=2, space="PSUM") as psq:
                # PE warmup: ~4us of dependency-free matmuls on garbage data
                # unlock the HAM clock gate (K=8/8) while input DMAs run, so
                # the projections execute at 2.4 GHz instead of 1.2.
                nc.vector.memset(scr, 0.0)
                for _ in range(28):
                    wps = psq.tile([128, 512], f32, tag="pk", name="wps")
                    nc.tensor.matmul(wps, lhsT=scr[:, 0:128], rhs=scr[:, 128:640],
                                     start=True, stop=True)
                # critical-path loads first, spread across the 3 DMA queues.
                nc.gpsimd.dma_start(out=wk, in_=wk_d.rearrange("(k p) e -> p k e", p=128))
                for kc in range(3):
                    nc.sync.dma_start(out=xT[:, kc, :], in_=xT_r[:, kc, :])
                for kc in range(3, 6):
                    nc.scalar.dma_start(out=xT[:, kc, :], in_=xT_r[:, kc, :])
                nc.scalar.dma_start(out=wq, in_=wq_r)
                for kc in range(6, KCT):
                    nc.gpsimd.dma_start(out=xT[:, kc, :], in_=xT_r[:, kc, :])
                nc.gpsimd.dma_start(out=wv, in_=wv_d.rearrange("(k p) e -> p k e", p=128))
                nc.gpsimd.dma_start(
                    out=mtab[:, : _MT_OFF[4]], in_=mt_d[:, : _MT_OFF[4]])
                nc.vector.memset(v_sb[:, :, 64], 1.0)
                nc.vector.memset(v_sb[:, :, 129], 1.0)

                # k projection -> kRep (kv0 low half slot0, kv1 high slot1)
                for sc in range(4):
                    ps = psq.tile([128, 512], f32, tag="pk", name="kps")
                    for kc in range(KCT):
                        nc.tensor.matmul(
                            ps, lhsT=wk[:, kc, :],
                            rhs=xT[:, kc, 512 * sc:512 * (sc + 1)],
                            start=(kc == 0), stop=(kc == KCT - 1))
                    sl = slice(512 * sc, 512 * (sc + 1))
                    nc.vector.tensor_copy(kRep[0:64, 0, sl], ps[0:64, :])
                    nc.vector.tensor_copy(kRep[64:128, 1, sl], ps[64:128, :])
                    nc.sync.dma_start(out=kRep[64:128, 0, sl], in_=kRep[0:64, 0, sl])
                    nc.sync.dma_start(out=kRep[0:64, 1, sl], in_=kRep[64:128, 1, sl])

                # q pair 0 (psq pool; pairs 1-3 are emitted inside the heads)
                for tcn in range(4):
                    ps = psq.tile([128, 512], f32, tag="pk", name="q0ps")
                    for kc in range(KCT):
                        nc.tensor.matmul(
                            ps, lhsT=wq[:, kc, 0:128],
                            rhs=xT[:, kc, 512 * tcn:512 * (tcn + 1)],
                            start=(kc == 0), stop=(kc == KCT - 1))
                    sl = slice(512 * tcn, 512 * (tcn + 1))
                    nc.vector.tensor_copy(qRep[0:64, 0, sl], ps[0:64, :])
                    nc.vector.tensor_copy(qRep[64:128, 1, sl], ps[64:128, :])
                    nc.sync.dma_start(out=qRep[64:128, 0, sl],
                                      in_=qRep[0:64, 0, sl])
                    nc.sync.dma_start(out=qRep[0:64, 1, sl],
                                      in_=qRep[64:128, 1, sl])

              # psA opened first -> banks 0-3 (WAR only on the k/q0 psq
              # tiles, which complete before the first attnv); psS -> 4-7.
              with (
                tc.tile_pool(name="psA", bufs=1, space="PSUM") as psA,
                tc.tile_pool(name="psS", bufs=2, space="PSUM") as psS,
              ):
                def q_chunk(p, tcn):
                    ps = psS.tile([128, 512], f32, tag="S", name="qps")
                    for kc in range(KCT):
                        nc.tensor.matmul(
                            ps, lhsT=wq[:, kc, 128 * p:128 * (p + 1)],
                            rhs=xT[:, kc, 512 * tcn:512 * (tcn + 1)],
                            start=(kc == 0), stop=(kc == KCT - 1))
                    sl = slice(512 * tcn, 512 * (tcn + 1))
                    nc.vector.tensor_copy(qRep[0:64, 2 * p, sl], ps[0:64, :])
                    nc.vector.tensor_copy(qRep[64:128, 2 * p + 1, sl],
                                          ps[64:128, :])
                    nc.sync.dma_start(out=qRep[64:128, 2 * p, sl],
                                      in_=qRep[0:64, 2 * p, sl])
                    nc.sync.dma_start(out=qRep[0:64, 2 * p + 1, sl],
                                      in_=qRep[64:128, 2 * p + 1, sl])

                def v_chunk(st):
                    ps = psS.tile([128, 128], f32, tag="S", name="vps")
                    for kc in range(KCT):
                        nc.tensor.matmul(
                            ps, lhsT=xT[:, kc, 128 * st:128 * (st + 1)],
                            rhs=wv[:, kc, :],
                            start=(kc == 0), stop=(kc == KCT - 1))
                    nc.vector.tensor_copy(v_sb[:, st, 0:64], ps[:, 0:64])
                    nc.vector.tensor_copy(v_sb[:, st, 65:129], ps[:, 64:128])

                # insertion schedule: (head, j) -> projection jobs. v feeds
                # attnv(0,j) just-in-time during head 0 (nearly free there);
                # q-pair p's 4 chunks land where the ACT is widest.
                ins = {}
                for st in range(ST):
                    ins.setdefault((0, st), []).append(("v", st))
                for tcn in range(4):
                    ins.setdefault((1, 2 + 4 * tcn), []).append(("q", 1, tcn))
                for p_ in (2, 3):
                    for tcn in range(4):
                        h_ = 2 * (p_ - 1) + tcn // 2
                        ins.setdefault((h_, 4 + 8 * (tcn % 2)), []).append(
                            ("q", p_, tcn))

                pending = None   # (i, E, j) awaiting TT + attnv
                head_pa = {}     # head -> psum accumulator, allocated lazily

                def flush_pending():
                    nonlocal pending
                    if pending is None:
                        return
                    fi, fE, fj = pending
                    if fi not in head_pa:
                        head_pa[fi] = psA.tile([65, T], f32, tag="pa",
                                               name=f"pa_h{fi}")
                    fpa = head_pa[fi]
                    fhalf = fi % 2
                    W = _W[fi][fj]
                    lo = 128 * fj
                    nch = (W + 511) // 512
                    # alibi decay for the diag+past region [lo, W)
                    nc.vector.tensor_tensor(
                        fE[:, lo:W], fE[:, lo:W],
                        mtab[:, _MT_OFF[fi]:_MT_OFF[fi] + (W - lo)], MUL)
                    for tcn in range(nch):
                        w_c = min(512, W - 512 * tcn)
                        nc.tensor.matmul(
                            fpa[:, 512 * tcn:512 * tcn + w_c],
                            lhsT=v_sb[:, fj, 65 * fhalf:65 * fhalf + 65],
                            rhs=fE[:, 512 * tcn:512 * tcn + w_c],
                            start=(fj == _J_FIRST[fi][tcn]),
                            stop=(fj == ST - 1),
                            skip_group_check=True)
                    pending = None
                    if fj == ST - 1:
                        emit_head_epilogue(fi, fpa)
                        del head_pa[fi]

                def emit_head_epilogue(i, pa):
                    p, half = i // 2, i % 2
                    # copy-out: rows 0:64 -> outT half; row 64 -> denom.
                    # For the last head, split the copy across DVE and ACT and
                    # move the outT DMA to the idle gpsimd queue so the
                    # denominator chain (sync queue) isn't delayed behind it.
                    st65 = dstgp.tile([65, T], bf16, tag="st65")
                    if i == NH - 1:
                        nc.vector.tensor_copy(st65[:, 0:1024], pa[0:65, 0:1024])
                        nc.scalar.copy(st65[:, 1024:T], pa[0:65, 1024:T])
                        # denominator chain first on sync; outT via gpsimd
                        nc.sync.dma_start(out=ddrow[i:i + 1, :],
                                          in_=st65[64:65, :])
                        nc.sync.dma_start(
                            out=dstack[16 * i:16 * (i + 1), :],
                            in_=ddrow[i].rearrange("(a b) -> a b", b=128))
                        nc.gpsimd.dma_start(
                            out=outT[64 * half:64 * half + 64, p, :],
                            in_=st65[0:64, :])
                    else:
                        nc.vector.tensor_copy(st65, pa[0:65, :])
                        nc.sync.dma_start(
                            out=outT[64 * half:64 * half + 64, p, :],
                            in_=st65[0:64, :])
                        nc.sync.dma_start(out=ddrow[i:i + 1, :],
                                          in_=st65[64:65, :])
                        nc.sync.dma_start(
                            out=dstack[16 * i:16 * (i + 1), :],
                            in_=ddrow[i].rearrange("(a b) -> a b", b=128))
                    if half == 1:
                        # pair p complete: reciprocal + normalize (DVE slices
                        # must be 32-partition aligned -> per pair, not head)
                        rsl = slice(32 * p, 32 * p + 32)
                        nc.vector.tensor_copy(dstf[rsl], dstack[rsl])
                        nc.vector.reciprocal(rstf[rsl], dstf[rsl])
                        nc.vector.tensor_copy(rstb[rsl], rstf[rsl])
                        rrep = work.tile([128, T], bf16, tag="rrep")
                        for hh in range(2):
                            ii = 2 * p + hh
                            nc.sync.dma_start(out=rd3[ii],
                                              in_=rstb[16 * ii:16 * (ii + 1), :])
                        for hh in range(2):
                            ii = 2 * p + hh
                            src = rdram[ii:ii + 1, :]
                            src = bass.AP(tensor=src.tensor, offset=src.offset,
                                          ap=[[0, 64]] + list(src.ap)[1:])
                            # same queue as the rd3 writes: the hand-built
                            # broadcast AP is ordered only by DMA-queue FIFO.
                            nc.sync.dma_start(out=rrep[64 * hh:64 * hh + 64, :],
                                              in_=src)
                        # chunked so phase-3 tiles can start as soon as their
                        # column range is normalized
                        for cc in range(4):
                            csl = slice(512 * cc, 512 * (cc + 1))
                            nc.vector.tensor_tensor(outT[:, p, csl],
                                                    outT[:, p, csl],
                                                    rrep[:, csl], MUL)

                for i in range(NH):
                    p, half = i // 2, i % 2
                    # just-in-time loads off the critical path
                    if 2 <= i < 6:
                        w_i = i + 2   # big-head mtab slices, two heads ahead
                        nc.gpsimd.dma_start(
                            out=mtab[:, _MT_OFF[w_i]:_MT_OFF[w_i] + _MT_W[w_i]],
                            in_=mt_d[:, _MT_OFF[w_i]:_MT_OFF[w_i] + _MT_W[w_i]])
                    if i == 2:
                        nc.gpsimd.dma_start(
                            out=wp, in_=wp_d.rearrange("(k p) e -> p k e", p=128))
                    for j in range(ST):
                        for job in ins.get((i, j), []):
                            if job[0] == "v":
                                v_chunk(job[1])
                            else:
                                q_chunk(job[1], job[2])
                        W = _W[i][j]
                        nch = (W + 511) // 512
                        E = ebufp.tile([128, T], bf16, tag="E")
                        # software pipeline: emit scores+ACT for (i,j) before
                        # TT+attnv of the previous iteration, so the next ACT
                        # never waits behind attnv in the tensor queue.
                        for sh in range(2):
                            c0, c1 = 2 * sh, min(nch, 2 * sh + 2)
                            if c0 >= c1:
                                continue
                            wv_ = min(W, 1024 * (sh + 1)) - 1024 * sh
                            S = psS.tile([128, 1024], f32, tag="S")
                            for tcn in range(c0, c1):
                                rh = 64 * (tcn % 2)
                                o = 512 * (tcn - c0)
                                w_c = min(512, W - 512 * tcn)
                                nc.tensor.matmul(
                                    S[:, o:o + w_c],
                                    lhsT=kRep[rh:rh + 64, half,
                                              128 * j:128 * (j + 1)],
                                    rhs=qRep[rh:rh + 64, i,
                                             512 * tcn:512 * tcn + w_c],
                                    start=True, stop=True)
                            nc.scalar.activation(
                                E[:, 1024 * sh:1024 * sh + wv_], S[:, :wv_],
                                AF.Exp, scale=0.125)
                        flush_pending()
                        pending = (i, E, j)
                flush_pending()

            # ---- phase 3: output projection ----
            with (
                tc.tile_pool(name="outp", bufs=4) as outp,
                tc.tile_pool(name="psPin", bufs=1, space="PSUM") as psPin,
                tc.tile_pool(name="psP", bufs=3, space="PSUM") as psP,
            ):
                # pin: grab the old psA banks (still draining the last pa
                # copy-out) with an unused tile so psP lands on the banks the
                # last ACT freed earlier; plus PE keep-warm matmuls so the
                # projection doesn't start at 1.2 GHz after the tail gap.
                pin = psPin.tile([65, T], f32)
                nc.vector.memset(pin[:, 0:16], 0.0)
                warm = psPin.tile([128, 512], f32, tag="warm")
                for _ in range(36):
                    nc.tensor.matmul(warm, lhsT=outT[:, 0, 0:128],
                                     rhs=outT[:, 0, 0:512], start=True, stop=True)
                dq = [nc.sync, nc.gpsimd, nc.scalar]
                for tt in range(ST):
                    osb = outp.tile([128, C], f32, tag="osb")
                    for ec in range(2):
                        ps = psP.tile([128, 512], f32, tag="pp")
                        for kt in range(4):
                            nc.tensor.matmul(
                                ps, lhsT=outT[:, kt, 128 * tt:128 * (tt + 1)],
                                rhs=wp[:, kt, 512 * ec:512 * (ec + 1)],
                                start=(kt == 0), stop=(kt == 3))
                        # split psum->sbuf copies across DVE and ACT so the
                        # copy never gates the next psP tile
                        if ec == 0:
                            nc.vector.tensor_copy(
                                osb[:, 512 * ec:512 * (ec + 1)], ps)
                        else:
                            nc.scalar.copy(osb[:, 512 * ec:512 * (ec + 1)], ps)
                    dq[tt % 3].dma_start(out=out_d[128 * tt:128 * (tt + 1), :],
                                         in_=osb)

    _split_multiwait(nc, mybir)
    _NC_CACHE["nc"] = nc
    return nc


def _prep_core_inputs(x, Wq, Wkv, Wproj, b, g):
    import ml_dtypes
    bf = ml_dtypes.bfloat16
    heads = [_head_of_slot(i, g) for i in range(NH)]
    xT = np.ascontiguousarray(x[b].T).astype(bf)                      # [C, T]
    wq_cols = np.concatenate([Wq[64 * h:64 * (h + 1)] for h in heads], axis=0)
    wqT = np.ascontiguousarray(wq_cols.T).astype(bf)                  # [C, 512]
    wkT = np.ascontiguousarray(Wkv[128 * g:128 * (g + 1)].T).astype(bf)
    wvT = np.ascontiguousarray(Wkv[256 + 128 * g:256 + 128 * (g + 1)].T).astype(bf)
    cols = np.concatenate([np.arange(64 * h, 64 * (h + 1)) for h in heads])
    wpT = np.ascontiguousarray(Wproj[:, cols].T).astype(bf)           # [512, C]

    s_in = np.arange(128, dtype=np.float64)
    mtab = np.zeros((128, _MT_TOT), dtype=bf)
    for i, h in enumerate(heads):
        a = _a_of_head(h)
        d = np.arange(_MT_W[i], dtype=np.float64)
        m = np.minimum(np.exp(-a * (d[None, :] - s_in[:, None])), 1.0)
        mtab[:, _MT_OFF[i]:_MT_OFF[i] + _MT_W[i]] = m.astype(np.float32)
    return {"xT": xT, "wqT": wqT, "wkT": wkT, "wvT": wvT, "wpT": wpT,
            "mtab": mtab}


def kernel(x, Wq, Wkv, Wproj, bproj):
    from concourse.bass_utils import run_bass_kernel_spmd
    x = np.asarray(x, dtype=np.float32)
    Wq = np.asarray(Wq, dtype=np.float32)
    Wkv = np.asarray(Wkv, dtype=np.float32)
    Wproj = np.asarray(Wproj, dtype=np.float32)
    bproj = np.asarray(bproj, dtype=np.float32)

    nc = _build_nc()
    in_maps = [_prep_core_inputs(x, Wq, Wkv, Wproj, c // 2, c % 2)
               for c in range(8)]
    res = run_bass_kernel_spmd(nc, in_maps, core_ids=list(range(8)))
    out = np.zeros((B, T, C), dtype=np.float32)
    for c in range(8):
        out[c // 2] += res.results[c]["out"]
    out += bproj[None, None, :]
    return out
